# revision 7
# baseline (speedup 1.0000x reference)
"""Trainium2 Bass kernel for nn_Discriminator_15668040696127.

Computes:
    q, a, d = samples[:, 0], samples[:, 1], samples[:, 2]        # [B, D]
    cos1 = <q,d> / max(||q||*||d||, 1e-6)                         # [B]
    cos2 = <a,d> / max(||a||*||d||, 1e-6)                         # [B]
    score = cos1 @ D_v1 + cos2 @ D_v2                             # scalar
    out = BCE_with_logits(score, labels[0])                       # scalar

Sharding: data-parallel over B across 8 NeuronCores (1024 samples each).
Each core computes a partial score; an on-device AllReduce sums them and
every core evaluates the (scalar) BCE; the host reads core 0's output.
"""

import os
import sys

import numpy as np

for _p in ("/opt/trn_rl_repo", "/root/.axon_site/_ro/trn_rl_repo"):
    if os.path.isdir(_p) and _p not in sys.path:
        sys.path.append(_p)

import concourse.bass as bass
import concourse.bacc as bacc
import concourse.mybir as mybir
import concourse.tile as tile
from concourse import bass_utils

N_CORES = 8
B, D = 8192, 4096
BS = B // N_CORES          # 1024 samples per core
P = 128                    # SBUF partitions
T = BS // P                # 8 tiles of 128 samples per core
EPS = 1e-6

f32 = mybir.dt.float32
Alu = mybir.AluOpType
Act = mybir.ActivationFunctionType

_CACHE = {}


def _build_program():
    nc = bacc.Bacc(
        "TRN2",
        target_bir_lowering=False,
        debug=False,
        num_devices=N_CORES,
    )

    samples = nc.dram_tensor("samples", [BS, 3, D], f32, kind="ExternalInput")
    labels = nc.dram_tensor("labels", [1], f32, kind="ExternalInput")
    dv1 = nc.dram_tensor("dv1", [BS], f32, kind="ExternalInput")
    dv2 = nc.dram_tensor("dv2", [BS], f32, kind="ExternalInput")
    out = nc.dram_tensor("out", [1, 1], f32, kind="ExternalOutput")

    with tile.TileContext(nc) as tc:
        with (
            tc.tile_pool(name="data", bufs=3) as data_pool,
            tc.tile_pool(name="junk", bufs=1) as junk_pool,
            tc.tile_pool(name="stats", bufs=1) as stats_pool,
            tc.tile_pool(name="psum", bufs=1, space="PSUM") as psum_pool,
            tc.tile_pool(name="dram", bufs=1, space="DRAM") as dram_pool,
        ):
            # Per-sample statistics, one column per 128-sample tile.
            qd_s = stats_pool.tile([P, T], f32, tag="qd")
            ad_s = stats_pool.tile([P, T], f32, tag="ad")
            qq_s = stats_pool.tile([P, T], f32, tag="qq")
            aa_s = stats_pool.tile([P, T], f32, tag="aa")
            dd_s = stats_pool.tile([P, T], f32, tag="dd")

            for t in range(T):
                # One contiguous 6 MB DMA: 128 samples x (q,a,d) x 4096 f32.
                dat = data_pool.tile([P, 3, D], f32, tag="dat")
                nc.sync.dma_start(dat[:], samples[bass.ts(t, P), :, :])
                q = dat[:, 0, :]
                a = dat[:, 1, :]
                d = dat[:, 2, :]

                # DVE: fused product + per-partition accumulate
                # (scalar_tensor_tensor; accum_out must be a standalone
                # tile — strided accum destinations crash the HW).
                for src0, src1, dst, atag in (
                    (q, d, qd_s, "qd1"),
                    (a, d, ad_s, "ad1"),
                    (d, d, dd_s, "dd1"),
                ):
                    jd = junk_pool.tile([P, D], f32, tag="junk_dve")
                    acc = junk_pool.tile([P, 1], f32, tag=atag)
                    nc.vector.scalar_tensor_tensor(
                        out=jd[:], in0=src0, scalar=1.0, in1=src1,
                        op0=Alu.mult, op1=Alu.mult, accum_out=acc[:],
                    )
                    nc.vector.tensor_copy(dst[:, t : t + 1], acc[:])

                # ACT: square + accumulate for the q/a norms
                # (copies stay on ACT to avoid cross-engine syncs).
                for src0, dst, atag in ((q, qq_s, "qq1"), (a, aa_s, "aa1")):
                    ja = junk_pool.tile([P, D], f32, tag="junk_act")
                    acc = junk_pool.tile([P, 1], f32, tag=atag)
                    nc.scalar.activation(
                        out=ja[:], in_=src0, func=Act.Square, accum_out=acc[:],
                    )
                    nc.scalar.copy(dst[:, t : t + 1], acc[:])

            # cos = dot / max(sqrt(n1*n2), EPS), all on [128, T] stats.
            # sqrt(v) = exp(0.5*ln(v)) keeps the whole kernel on the
            # natural_log_exp activation table (no table reload).
            small = stats_pool.tile([P, T], f32, tag="small0")
            inv1 = stats_pool.tile([P, T], f32, tag="inv1")
            nc.vector.tensor_mul(small[:], qq_s[:], dd_s[:])
            nc.scalar.activation(small[:], small[:], Act.Ln)
            nc.scalar.activation(small[:], small[:], Act.Exp, scale=0.5)
            nc.vector.tensor_scalar_max(small[:], small[:], EPS)
            nc.vector.reciprocal(inv1[:], small[:])

            small2 = stats_pool.tile([P, T], f32, tag="small2")
            inv2 = stats_pool.tile([P, T], f32, tag="inv2")
            nc.vector.tensor_mul(small2[:], aa_s[:], dd_s[:])
            nc.scalar.activation(small2[:], small2[:], Act.Ln)
            nc.scalar.activation(small2[:], small2[:], Act.Exp, scale=0.5)
            nc.vector.tensor_scalar_max(small2[:], small2[:], EPS)
            nc.vector.reciprocal(inv2[:], small2[:])

            cos1 = stats_pool.tile([P, T], f32, tag="cos1")
            cos2 = stats_pool.tile([P, T], f32, tag="cos2")
            nc.vector.tensor_mul(cos1[:], qd_s[:], inv1[:])
            nc.vector.tensor_mul(cos2[:], ad_s[:], inv2[:])

            # Weight by D_v1/D_v2 (laid out [p, t] to match the stats tiles).
            dv1_t = stats_pool.tile([P, T], f32, tag="dv1")
            dv2_t = stats_pool.tile([P, T], f32, tag="dv2")
            nc.sync.dma_start(dv1_t[:], dv1[:].rearrange("(n p) -> p n", p=P))
            nc.sync.dma_start(dv2_t[:], dv2[:].rearrange("(n p) -> p n", p=P))

            contrib = stats_pool.tile([P, T], f32, tag="contrib")
            contrib2 = stats_pool.tile([P, T], f32, tag="contrib2")
            nc.vector.tensor_mul(contrib[:], cos1[:], dv1_t[:])
            nc.vector.tensor_mul(contrib2[:], cos2[:], dv2_t[:])
            nc.vector.tensor_add(contrib[:], contrib[:], contrib2[:])

            row_sum = stats_pool.tile([P, 1], f32, tag="row_sum")
            nc.vector.reduce_sum(row_sum[:], contrib[:], axis=mybir.AxisListType.X)

            # Partition reduction via PE: [1,1] = row_sum^T @ ones.
            ones = stats_pool.tile([P, 1], f32, tag="ones")
            nc.gpsimd.memset(ones[:], 1.0)
            psum_t = psum_pool.tile([1, 1], f32, tag="psum_s")
            nc.tensor.matmul(psum_t[:], row_sum[:], ones[:], start=True, stop=True)

            # Stage the partial score, AllReduce across the 8 cores.
            partial = stats_pool.tile([1, 8], f32, tag="partial")
            nc.gpsimd.memset(partial[:], 0.0)
            nc.vector.tensor_copy(partial[0:1, 0:1], psum_t[:])

            # AllGather (lower floor than AllReduce), then sum the 8
            # per-core partials locally. AG concatenates on the
            # partition axis: cc_out row r = rank r's [1, 8] buffer.
            cc_in = dram_pool.tile([1, 8], f32, tag="cc_in")
            cc_out = dram_pool.tile([N_CORES, 8], f32, tag="cc_out")
            nc.sync.dma_start(cc_in[:], partial[:])
            nc.gpsimd.collective_compute(
                "AllGather",
                Alu.bypass,
                replica_groups=[list(range(N_CORES))],
                ins=[cc_in[:].opt()],
                outs=[cc_out[:].opt()],
            )
            gath = stats_pool.tile([1, N_CORES], f32, tag="gath")
            nc.sync.dma_start(gath[:], cc_out[:, 0:1].rearrange("r k -> k r"))
            red = stats_pool.tile([1, 1], f32, tag="red")
            nc.vector.reduce_sum(red[:], gath[:], axis=mybir.AxisListType.X)
            s = red[0:1, 0:1]

            # BCE with logits: max(s,0) - s*y + softplus(-|s|), on [1,1].
            ltile = stats_pool.tile([1, 1], f32, tag="ltile")
            nc.sync.dma_start(ltile[:], labels[None, :])

            relu_t = stats_pool.tile([1, 1], f32, tag="relu_t")
            abs_t = stats_pool.tile([1, 1], f32, tag="abs_t")
            exp_t = stats_pool.tile([1, 1], f32, tag="exp_t")
            sp_t = stats_pool.tile([1, 1], f32, tag="sp_t")
            xy_t = stats_pool.tile([1, 1], f32, tag="xy_t")
            bce_t = stats_pool.tile([1, 1], f32, tag="bce_t")
            nc.scalar.activation(relu_t[:], s, Act.Relu)
            nc.scalar.activation(abs_t[:], s, Act.Abs)
            # softplus(-|s|) = ln(1 + exp(-|s|)); Softplus has no HW table.
            nc.scalar.activation(exp_t[:], abs_t[:], Act.Exp, scale=-1.0)
            nc.scalar.activation(sp_t[:], exp_t[:], Act.Ln, bias=1.0)
            nc.vector.tensor_mul(xy_t[:], s, ltile[:])
            nc.vector.tensor_sub(bce_t[:], relu_t[:], xy_t[:])
            nc.vector.tensor_add(bce_t[:], bce_t[:], sp_t[:])

            nc.sync.dma_start(out[:], bce_t[:])

    nc.compile()
    return nc


def _get_program():
    if "nc" not in _CACHE:
        _CACHE["nc"] = _build_program()
    return _CACHE["nc"]


def kernel(samples, labels, D_v1, D_v2):
    samples = np.asarray(samples, dtype=np.float32)
    labels = np.asarray(labels, dtype=np.float32)
    D_v1 = np.asarray(D_v1, dtype=np.float32)
    D_v2 = np.asarray(D_v2, dtype=np.float32)
    assert samples.shape == (B, 3, D), samples.shape

    nc = _get_program()

    in_maps = []
    for c in range(N_CORES):
        sl = slice(c * BS, (c + 1) * BS)
        in_maps.append(
            {
                "samples": np.ascontiguousarray(samples[sl]),
                "labels": labels,
                "dv1": np.ascontiguousarray(D_v1[sl]),
                "dv2": np.ascontiguousarray(D_v2[sl]),
            }
        )

    res = bass_utils.run_bass_kernel_spmd(nc, in_maps, core_ids=list(range(N_CORES)))
    _CACHE["last_results"] = res
    return np.asarray(res.results[0]["out"], dtype=np.float32).reshape(())


# revision 11
# speedup vs baseline: 1.0285x; 1.0285x over previous
"""Trainium2 Bass kernel for nn_Discriminator_15668040696127.

Computes:
    q, a, d = samples[:, 0], samples[:, 1], samples[:, 2]        # [B, D]
    cos1 = <q,d> / max(||q||*||d||, 1e-6)                         # [B]
    cos2 = <a,d> / max(||a||*||d||, 1e-6)                         # [B]
    score = cos1 @ D_v1 + cos2 @ D_v2                             # scalar
    out = BCE_with_logits(score, labels[0])                       # scalar

Sharding: data-parallel over B across 8 NeuronCores (1024 samples each).
Each core computes a partial score; an on-device AllReduce sums them and
every core evaluates the (scalar) BCE; the host reads core 0's output.
"""

import os
import sys

import numpy as np

for _p in ("/opt/trn_rl_repo", "/root/.axon_site/_ro/trn_rl_repo"):
    if os.path.isdir(_p) and _p not in sys.path:
        sys.path.append(_p)

import concourse.bass as bass
import concourse.bacc as bacc
import concourse.mybir as mybir
import concourse.tile as tile
from concourse import bass_utils

N_CORES = 8
B, D = 8192, 4096
BS = B // N_CORES          # 1024 samples per core
P = 128                    # SBUF partitions
T = BS // P                # 8 tiles of 128 samples per core
EPS = 1e-6

f32 = mybir.dt.float32
Alu = mybir.AluOpType
Act = mybir.ActivationFunctionType

_CACHE = {}


def _build_program():
    nc = bacc.Bacc(
        "TRN2",
        target_bir_lowering=False,
        debug=False,
        num_devices=N_CORES,
    )

    samples = nc.dram_tensor("samples", [BS, 3, D], f32, kind="ExternalInput")
    labels = nc.dram_tensor("labels", [1], f32, kind="ExternalInput")
    dv1 = nc.dram_tensor("dv1", [BS], f32, kind="ExternalInput")
    dv2 = nc.dram_tensor("dv2", [BS], f32, kind="ExternalInput")
    out = nc.dram_tensor("out", [1, 1], f32, kind="ExternalOutput")

    with tile.TileContext(nc) as tc:
        with (
            tc.tile_pool(name="data", bufs=3) as data_pool,
            tc.tile_pool(name="junk", bufs=1) as junk_pool,
            tc.tile_pool(name="stats", bufs=1) as stats_pool,
            tc.tile_pool(name="psum", bufs=1, space="PSUM") as psum_pool,
            tc.tile_pool(name="dram", bufs=1, space="DRAM") as dram_pool,
        ):
            # Per-sample statistics, one column per 128-sample tile.
            qd_s = stats_pool.tile([P, T], f32, tag="qd")
            ad_s = stats_pool.tile([P, T], f32, tag="ad")
            qq_s = stats_pool.tile([P, T], f32, tag="qq")
            aa_s = stats_pool.tile([P, T], f32, tag="aa")
            dd_s = stats_pool.tile([P, T], f32, tag="dd")

            # Small weight/label loads up front, off the critical tail.
            dv1_t = stats_pool.tile([P, T], f32, tag="dv1")
            dv2_t = stats_pool.tile([P, T], f32, tag="dv2")
            ltile = stats_pool.tile([1, 1], f32, tag="ltile")
            nc.sync.dma_start(dv1_t[:], dv1[:].rearrange("(n p) -> p n", p=P))
            nc.sync.dma_start(dv2_t[:], dv2[:].rearrange("(n p) -> p n", p=P))
            nc.sync.dma_start(ltile[:], labels[None, :])

            for t in range(T):
                # Three 2 MB DMAs (d first) so compute can start as soon
                # as each component lands, not after the whole 6 MB tile.
                d_t = data_pool.tile([P, D], f32, tag="d")
                q_t = data_pool.tile([P, D], f32, tag="q")
                a_t = data_pool.tile([P, D], f32, tag="a")
                nc.sync.dma_start(d_t[:], samples[bass.ts(t, P), 2, :])
                nc.sync.dma_start(q_t[:], samples[bass.ts(t, P), 0, :])
                nc.sync.dma_start(a_t[:], samples[bass.ts(t, P), 1, :])
                q, a, d = q_t[:], a_t[:], d_t[:]

                # DVE: fused product + per-partition accumulate
                # (scalar_tensor_tensor; accum_out must be a standalone
                # tile — strided accum destinations crash the HW).
                for src0, src1, dst, atag in (
                    (d, d, dd_s, "dd1"),
                    (q, d, qd_s, "qd1"),
                    (a, d, ad_s, "ad1"),
                ):
                    jd = junk_pool.tile([P, D], f32, tag="junk_dve")
                    acc = junk_pool.tile([P, 1], f32, tag=atag)
                    nc.vector.scalar_tensor_tensor(
                        out=jd[:], in0=src0, scalar=1.0, in1=src1,
                        op0=Alu.mult, op1=Alu.mult, accum_out=acc[:],
                    )
                    nc.vector.tensor_copy(dst[:, t : t + 1], acc[:])

                # ACT: square + accumulate for the q/a norms
                # (copies stay on ACT to avoid cross-engine syncs).
                for src0, dst, atag in ((q, qq_s, "qq1"), (a, aa_s, "aa1")):
                    ja = junk_pool.tile([P, D], f32, tag="junk_act")
                    acc = junk_pool.tile([P, 1], f32, tag=atag)
                    nc.scalar.activation(
                        out=ja[:], in_=src0, func=Act.Square, accum_out=acc[:],
                    )
                    nc.scalar.copy(dst[:, t : t + 1], acc[:])

            # cos = dot / max(sqrt(n1*n2), EPS), all on [128, T] stats.
            # sqrt(v) = exp(0.5*ln(v)) keeps the whole kernel on the
            # natural_log_exp activation table (no table reload).
            small = stats_pool.tile([P, T], f32, tag="small0")
            inv1 = stats_pool.tile([P, T], f32, tag="inv1")
            nc.vector.tensor_mul(small[:], qq_s[:], dd_s[:])
            nc.scalar.activation(small[:], small[:], Act.Ln)
            nc.scalar.activation(small[:], small[:], Act.Exp, scale=0.5)
            nc.vector.tensor_scalar_max(small[:], small[:], EPS)
            nc.vector.reciprocal(inv1[:], small[:])

            small2 = stats_pool.tile([P, T], f32, tag="small2")
            inv2 = stats_pool.tile([P, T], f32, tag="inv2")
            nc.vector.tensor_mul(small2[:], aa_s[:], dd_s[:])
            nc.scalar.activation(small2[:], small2[:], Act.Ln)
            nc.scalar.activation(small2[:], small2[:], Act.Exp, scale=0.5)
            nc.vector.tensor_scalar_max(small2[:], small2[:], EPS)
            nc.vector.reciprocal(inv2[:], small2[:])

            cos1 = stats_pool.tile([P, T], f32, tag="cos1")
            cos2 = stats_pool.tile([P, T], f32, tag="cos2")
            nc.vector.tensor_mul(cos1[:], qd_s[:], inv1[:])
            nc.vector.tensor_mul(cos2[:], ad_s[:], inv2[:])

            contrib = stats_pool.tile([P, T], f32, tag="contrib")
            contrib2 = stats_pool.tile([P, T], f32, tag="contrib2")
            nc.vector.tensor_mul(contrib[:], cos1[:], dv1_t[:])
            nc.vector.tensor_mul(contrib2[:], cos2[:], dv2_t[:])
            nc.vector.tensor_add(contrib[:], contrib[:], contrib2[:])

            row_sum = stats_pool.tile([P, 1], f32, tag="row_sum")
            nc.vector.reduce_sum(row_sum[:], contrib[:], axis=mybir.AxisListType.X)

            # Partition reduction via PE: [1,1] = row_sum^T @ ones.
            ones = stats_pool.tile([P, 1], f32, tag="ones")
            nc.gpsimd.memset(ones[:], 1.0)
            psum_t = psum_pool.tile([1, 1], f32, tag="psum_s")
            nc.tensor.matmul(psum_t[:], row_sum[:], ones[:], start=True, stop=True)

            # Stage the partial score, AllReduce across the 8 cores.
            partial = stats_pool.tile([1, 8], f32, tag="partial")
            nc.gpsimd.memset(partial[:], 0.0)
            nc.vector.tensor_copy(partial[0:1, 0:1], psum_t[:])

            cc_in = dram_pool.tile([1, 8], f32, tag="cc_in")
            cc_out = dram_pool.tile([1, 8], f32, tag="cc_out")
            nc.sync.dma_start(cc_in[:], partial[:])
            nc.gpsimd.collective_compute(
                "AllReduce",
                Alu.add,
                replica_groups=[list(range(N_CORES))],
                ins=[cc_in[:].opt()],
                outs=[cc_out[:].opt()],
            )
            red = stats_pool.tile([1, 8], f32, tag="red")
            nc.sync.dma_start(red[:], cc_out[:])
            s = red[0:1, 0:1]

            # BCE with logits: max(s,0) - s*y + softplus(-|s|), on [1,1].
            relu_t = stats_pool.tile([1, 1], f32, tag="relu_t")
            abs_t = stats_pool.tile([1, 1], f32, tag="abs_t")
            exp_t = stats_pool.tile([1, 1], f32, tag="exp_t")
            sp_t = stats_pool.tile([1, 1], f32, tag="sp_t")
            xy_t = stats_pool.tile([1, 1], f32, tag="xy_t")
            bce_t = stats_pool.tile([1, 1], f32, tag="bce_t")
            nc.scalar.activation(relu_t[:], s, Act.Relu)
            nc.scalar.activation(abs_t[:], s, Act.Abs)
            # softplus(-|s|) = ln(1 + exp(-|s|)); Softplus has no HW table.
            nc.scalar.activation(exp_t[:], abs_t[:], Act.Exp, scale=-1.0)
            nc.scalar.activation(sp_t[:], exp_t[:], Act.Ln, bias=1.0)
            nc.vector.tensor_mul(xy_t[:], s, ltile[:])
            nc.vector.tensor_sub(bce_t[:], relu_t[:], xy_t[:])
            nc.vector.tensor_add(bce_t[:], bce_t[:], sp_t[:])

            nc.sync.dma_start(out[:], bce_t[:])

    nc.compile()
    return nc


def _get_program():
    if "nc" not in _CACHE:
        _CACHE["nc"] = _build_program()
    return _CACHE["nc"]


def kernel(samples, labels, D_v1, D_v2):
    samples = np.asarray(samples, dtype=np.float32)
    labels = np.asarray(labels, dtype=np.float32)
    D_v1 = np.asarray(D_v1, dtype=np.float32)
    D_v2 = np.asarray(D_v2, dtype=np.float32)
    assert samples.shape == (B, 3, D), samples.shape

    nc = _get_program()

    in_maps = []
    for c in range(N_CORES):
        sl = slice(c * BS, (c + 1) * BS)
        in_maps.append(
            {
                "samples": np.ascontiguousarray(samples[sl]),
                "labels": labels,
                "dv1": np.ascontiguousarray(D_v1[sl]),
                "dv2": np.ascontiguousarray(D_v2[sl]),
            }
        )

    res = bass_utils.run_bass_kernel_spmd(nc, in_maps, core_ids=list(range(N_CORES)))
    _CACHE["last_results"] = res
    return np.asarray(res.results[0]["out"], dtype=np.float32).reshape(())


# revision 15
# speedup vs baseline: 1.0822x; 1.0523x over previous
"""Trainium2 Bass kernel for nn_Discriminator_15668040696127.

Computes:
    q, a, d = samples[:, 0], samples[:, 1], samples[:, 2]        # [B, D]
    cos1 = <q,d> / max(||q||*||d||, 1e-6)                         # [B]
    cos2 = <a,d> / max(||a||*||d||, 1e-6)                         # [B]
    score = cos1 @ D_v1 + cos2 @ D_v2                             # scalar
    out = BCE_with_logits(score, labels[0])                       # scalar

Sharding: data-parallel over B across 8 NeuronCores (1024 samples each).
Each core computes a partial score; an on-device AllReduce sums them and
every core evaluates the (scalar) BCE; the host reads core 0's output.
"""

import os
import sys

import numpy as np

for _p in ("/opt/trn_rl_repo", "/root/.axon_site/_ro/trn_rl_repo"):
    if os.path.isdir(_p) and _p not in sys.path:
        sys.path.append(_p)

import concourse.bass as bass
import concourse.bacc as bacc
import concourse.mybir as mybir
import concourse.tile as tile
from concourse import bass_utils

N_CORES = 8
B, D = 8192, 4096
BS = B // N_CORES          # 1024 samples per core
P = 128                    # SBUF partitions
T = BS // P                # 8 tiles of 128 samples per core
EPS = 1e-6

f32 = mybir.dt.float32
Alu = mybir.AluOpType
Act = mybir.ActivationFunctionType

_CACHE = {}


def _build_program():
    nc = bacc.Bacc(
        "TRN2",
        target_bir_lowering=False,
        debug=False,
        num_devices=N_CORES,
    )

    samples = nc.dram_tensor("samples", [BS, 3, D], f32, kind="ExternalInput")
    labels = nc.dram_tensor("labels", [1], f32, kind="ExternalInput")
    dv1 = nc.dram_tensor("dv1", [BS], f32, kind="ExternalInput")
    dv2 = nc.dram_tensor("dv2", [BS], f32, kind="ExternalInput")
    out = nc.dram_tensor("out", [1, 1], f32, kind="ExternalOutput")

    with tile.TileContext(nc) as tc:
        with (
            tc.tile_pool(name="data", bufs=3) as data_pool,
            tc.tile_pool(name="junk", bufs=1) as junk_pool,
            tc.tile_pool(name="stats", bufs=1) as stats_pool,
            tc.tile_pool(name="psum", bufs=1, space="PSUM") as psum_pool,
            tc.tile_pool(name="dram", bufs=1, space="DRAM") as dram_pool,
        ):
            # Per-sample statistics, one column per 128-sample tile.
            qd_s = stats_pool.tile([P, T], f32, tag="qd")
            ad_s = stats_pool.tile([P, T], f32, tag="ad")
            qq_s = stats_pool.tile([P, T], f32, tag="qq")
            aa_s = stats_pool.tile([P, T], f32, tag="aa")
            dd_s = stats_pool.tile([P, T], f32, tag="dd")

            # Small weight/label loads up front, off the critical tail.
            dv1_t = stats_pool.tile([P, T], f32, tag="dv1")
            dv2_t = stats_pool.tile([P, T], f32, tag="dv2")
            ltile = stats_pool.tile([1, 1], f32, tag="ltile")
            # SWDGE (gpsimd) keeps these descriptor-heavy small loads off
            # the HWDGE ring that streams the 2 MB sample tiles.
            nc.gpsimd.dma_start(dv1_t[:], dv1[:].rearrange("(n p) -> p n", p=P))
            nc.gpsimd.dma_start(dv2_t[:], dv2[:].rearrange("(n p) -> p n", p=P))
            nc.gpsimd.dma_start(ltile[:], labels[None, :])

            for t in range(T):
                # Three 2 MB DMAs (d first) so compute can start as soon
                # as each component lands, not after the whole 6 MB tile.
                d_t = data_pool.tile([P, D], f32, tag="d")
                q_t = data_pool.tile([P, D], f32, tag="q")
                a_t = data_pool.tile([P, D], f32, tag="a")
                nc.sync.dma_start(d_t[:], samples[bass.ts(t, P), 2, :])
                nc.sync.dma_start(q_t[:], samples[bass.ts(t, P), 0, :])
                nc.sync.dma_start(a_t[:], samples[bass.ts(t, P), 1, :])
                q, a, d = q_t[:], a_t[:], d_t[:]

                # DVE: fused product + per-partition accumulate
                # (scalar_tensor_tensor; accum_out must be a standalone
                # tile — strided accum destinations crash the HW).
                for src0, src1, dst, atag in (
                    (d, d, dd_s, "dd1"),
                    (q, d, qd_s, "qd1"),
                    (a, d, ad_s, "ad1"),
                ):
                    jd = junk_pool.tile([P, D], f32, tag="junk_dve")
                    acc = junk_pool.tile([P, 1], f32, tag=atag)
                    nc.vector.scalar_tensor_tensor(
                        out=jd[:], in0=src0, scalar=1.0, in1=src1,
                        op0=Alu.mult, op1=Alu.mult, accum_out=acc[:],
                    )
                    nc.vector.tensor_copy(dst[:, t : t + 1], acc[:])

                # ACT: square + accumulate for the q/a norms
                # (copies stay on ACT to avoid cross-engine syncs).
                for src0, dst, atag in ((q, qq_s, "qq1"), (a, aa_s, "aa1")):
                    ja = junk_pool.tile([P, D], f32, tag="junk_act")
                    acc = junk_pool.tile([P, 1], f32, tag=atag)
                    nc.scalar.activation(
                        out=ja[:], in_=src0, func=Act.Square, accum_out=acc[:],
                    )
                    nc.scalar.copy(dst[:, t : t + 1], acc[:])

            # cos = dot / max(sqrt(n1*n2), EPS), all on [128, T] stats.
            # sqrt(v) = exp(0.5*ln(v)) keeps the whole kernel on the
            # natural_log_exp activation table (no table reload).
            small = stats_pool.tile([P, T], f32, tag="small0")
            inv1 = stats_pool.tile([P, T], f32, tag="inv1")
            nc.vector.tensor_mul(small[:], qq_s[:], dd_s[:])
            nc.scalar.activation(small[:], small[:], Act.Ln)
            nc.scalar.activation(small[:], small[:], Act.Exp, scale=0.5)
            nc.vector.tensor_scalar_max(small[:], small[:], EPS)
            nc.vector.reciprocal(inv1[:], small[:])

            small2 = stats_pool.tile([P, T], f32, tag="small2")
            inv2 = stats_pool.tile([P, T], f32, tag="inv2")
            nc.vector.tensor_mul(small2[:], aa_s[:], dd_s[:])
            nc.scalar.activation(small2[:], small2[:], Act.Ln)
            nc.scalar.activation(small2[:], small2[:], Act.Exp, scale=0.5)
            nc.vector.tensor_scalar_max(small2[:], small2[:], EPS)
            nc.vector.reciprocal(inv2[:], small2[:])

            cos1 = stats_pool.tile([P, T], f32, tag="cos1")
            cos2 = stats_pool.tile([P, T], f32, tag="cos2")
            nc.vector.tensor_mul(cos1[:], qd_s[:], inv1[:])
            nc.vector.tensor_mul(cos2[:], ad_s[:], inv2[:])

            contrib = stats_pool.tile([P, T], f32, tag="contrib")
            contrib2 = stats_pool.tile([P, T], f32, tag="contrib2")
            nc.vector.tensor_mul(contrib[:], cos1[:], dv1_t[:])
            nc.vector.tensor_mul(contrib2[:], cos2[:], dv2_t[:])
            nc.vector.tensor_add(contrib[:], contrib[:], contrib2[:])

            row_sum = stats_pool.tile([P, 1], f32, tag="row_sum")
            nc.vector.reduce_sum(row_sum[:], contrib[:], axis=mybir.AxisListType.X)

            # Partition reduction via PE: [1,1] = row_sum^T @ ones.
            ones = stats_pool.tile([P, 1], f32, tag="ones")
            nc.gpsimd.memset(ones[:], 1.0)
            psum_t = psum_pool.tile([1, 1], f32, tag="psum_s")
            nc.tensor.matmul(psum_t[:], row_sum[:], ones[:], start=True, stop=True)

            # Stage the partial score, AllReduce across the 8 cores.
            partial = stats_pool.tile([1, 8], f32, tag="partial")
            nc.gpsimd.memset(partial[:], 0.0)
            nc.vector.tensor_copy(partial[0:1, 0:1], psum_t[:])

            cc_in = dram_pool.tile([1, 8], f32, tag="cc_in")
            cc_out = dram_pool.tile([1, 8], f32, tag="cc_out")
            nc.gpsimd.dma_start(cc_in[:], partial[:])
            nc.gpsimd.collective_compute(
                "AllReduce",
                Alu.add,
                replica_groups=[list(range(N_CORES))],
                ins=[cc_in[:].opt()],
                outs=[cc_out[:].opt()],
            )
            red = stats_pool.tile([1, 8], f32, tag="red")
            nc.sync.dma_start(red[:], cc_out[:])
            s = red[0:1, 0:1]

            # BCE with logits: max(s,0) - s*y + softplus(-|s|), on [1,1].
            relu_t = stats_pool.tile([1, 1], f32, tag="relu_t")
            abs_t = stats_pool.tile([1, 1], f32, tag="abs_t")
            exp_t = stats_pool.tile([1, 1], f32, tag="exp_t")
            sp_t = stats_pool.tile([1, 1], f32, tag="sp_t")
            xy_t = stats_pool.tile([1, 1], f32, tag="xy_t")
            bce_t = stats_pool.tile([1, 1], f32, tag="bce_t")
            # relu/abs on DVE: table ops on [1,1] cost ~1.3us on ACT.
            neg_t = stats_pool.tile([1, 1], f32, tag="neg_t")
            nc.vector.tensor_scalar_max(relu_t[:], s, 0.0)
            nc.vector.tensor_scalar_mul(neg_t[:], s, -1.0)
            nc.vector.tensor_max(abs_t[:], s, neg_t[:])
            # softplus(-|s|) = ln(1 + exp(-|s|)); Softplus has no HW table.
            nc.scalar.activation(exp_t[:], abs_t[:], Act.Exp, scale=-1.0)
            nc.scalar.activation(sp_t[:], exp_t[:], Act.Ln, bias=1.0)
            nc.vector.tensor_mul(xy_t[:], s, ltile[:])
            nc.vector.tensor_sub(bce_t[:], relu_t[:], xy_t[:])
            nc.vector.tensor_add(bce_t[:], bce_t[:], sp_t[:])

            nc.sync.dma_start(out[:], bce_t[:])

    nc.compile()
    return nc


def _get_program():
    if "nc" not in _CACHE:
        _CACHE["nc"] = _build_program()
    return _CACHE["nc"]


def kernel(samples, labels, D_v1, D_v2):
    samples = np.asarray(samples, dtype=np.float32)
    labels = np.asarray(labels, dtype=np.float32)
    D_v1 = np.asarray(D_v1, dtype=np.float32)
    D_v2 = np.asarray(D_v2, dtype=np.float32)
    assert samples.shape == (B, 3, D), samples.shape

    nc = _get_program()

    in_maps = []
    for c in range(N_CORES):
        sl = slice(c * BS, (c + 1) * BS)
        in_maps.append(
            {
                "samples": np.ascontiguousarray(samples[sl]),
                "labels": labels,
                "dv1": np.ascontiguousarray(D_v1[sl]),
                "dv2": np.ascontiguousarray(D_v2[sl]),
            }
        )

    res = bass_utils.run_bass_kernel_spmd(nc, in_maps, core_ids=list(range(N_CORES)))
    _CACHE["last_results"] = res
    return np.asarray(res.results[0]["out"], dtype=np.float32).reshape(())


# revision 17
# speedup vs baseline: 1.1155x; 1.0307x over previous
"""Trainium2 Bass kernel for nn_Discriminator_15668040696127.

Computes:
    q, a, d = samples[:, 0], samples[:, 1], samples[:, 2]        # [B, D]
    cos1 = <q,d> / max(||q||*||d||, 1e-6)                         # [B]
    cos2 = <a,d> / max(||a||*||d||, 1e-6)                         # [B]
    score = cos1 @ D_v1 + cos2 @ D_v2                             # scalar
    out = BCE_with_logits(score, labels[0])                       # scalar

Sharding: data-parallel over B across 8 NeuronCores (1024 samples each).
Each core computes a partial score; an on-device AllReduce sums them and
every core evaluates the (scalar) BCE; the host reads core 0's output.
"""

import os
import sys

import numpy as np

for _p in ("/opt/trn_rl_repo", "/root/.axon_site/_ro/trn_rl_repo"):
    if os.path.isdir(_p) and _p not in sys.path:
        sys.path.append(_p)

import concourse.bass as bass
import concourse.bacc as bacc
import concourse.mybir as mybir
import concourse.tile as tile
from concourse import bass_utils

N_CORES = 8
B, D = 8192, 4096
BS = B // N_CORES          # 1024 samples per core
P = 128                    # SBUF partitions
T = BS // P                # 8 tiles of 128 samples per core
EPS = 1e-6

f32 = mybir.dt.float32
Alu = mybir.AluOpType
Act = mybir.ActivationFunctionType

_CACHE = {}


def _build_program():
    nc = bacc.Bacc(
        "TRN2",
        target_bir_lowering=False,
        debug=False,
        num_devices=N_CORES,
    )

    samples = nc.dram_tensor("samples", [BS, 3, D], f32, kind="ExternalInput")
    labels = nc.dram_tensor("labels", [1], f32, kind="ExternalInput")
    dv1 = nc.dram_tensor("dv1", [BS], f32, kind="ExternalInput")
    dv2 = nc.dram_tensor("dv2", [BS], f32, kind="ExternalInput")
    out = nc.dram_tensor("out", [1, 1], f32, kind="ExternalOutput")

    with tile.TileContext(nc) as tc:
        with (
            tc.tile_pool(name="data", bufs=3) as data_pool,
            tc.tile_pool(name="junk", bufs=1) as junk_pool,
            tc.tile_pool(name="stats", bufs=1) as stats_pool,
            tc.tile_pool(name="psum", bufs=1, space="PSUM") as psum_pool,
            tc.tile_pool(name="dram", bufs=1, space="DRAM") as dram_pool,
        ):
            # dots[:, 0:T] = <q,d> columns, dots[:, T:2T] = <a,d>.
            # nprod[:, 0:T] = |q|^2*|d|^2, nprod[:, T:2T] = |a|^2*|d|^2.
            dots = stats_pool.tile([P, 2 * T], f32, tag="dots")
            nprod = stats_pool.tile([P, 2 * T], f32, tag="nprod")

            # Small weight/label loads up front, off the critical tail.
            # dvb[:, 0:T] = D_v1 laid out [p, t]; dvb[:, T:2T] = D_v2.
            dvb = stats_pool.tile([P, 2 * T], f32, tag="dvb")
            ltile = stats_pool.tile([1, 1], f32, tag="ltile")
            # SWDGE (gpsimd) keeps these descriptor-heavy small loads off
            # the HWDGE ring that streams the 2 MB sample tiles.
            nc.gpsimd.dma_start(dvb[:, 0:T], dv1[:].rearrange("(n p) -> p n", p=P))
            nc.gpsimd.dma_start(dvb[:, T : 2 * T], dv2[:].rearrange("(n p) -> p n", p=P))
            nc.gpsimd.dma_start(ltile[:], labels[None, :])

            # Warm-up collective: aligns core skew and wakes ncfw so the
            # real AllReduce at the tail pays a smaller entry latency.
            warm = stats_pool.tile([1, 8], f32, tag="warm")
            nc.gpsimd.memset(warm[:], 0.0)
            cc_w_in = dram_pool.tile([1, 8], f32, tag="cc_w_in")
            cc_w_out = dram_pool.tile([1, 8], f32, tag="cc_w_out")
            nc.gpsimd.dma_start(cc_w_in[:], warm[:])
            nc.gpsimd.collective_compute(
                "AllReduce",
                Alu.add,
                replica_groups=[list(range(N_CORES))],
                ins=[cc_w_in[:].opt()],
                outs=[cc_w_out[:].opt()],
            )

            for t in range(T):
                # Three 2 MB DMAs (d first) so compute can start as soon
                # as each component lands, not after the whole 6 MB tile.
                d_t = data_pool.tile([P, D], f32, tag="d")
                q_t = data_pool.tile([P, D], f32, tag="q")
                a_t = data_pool.tile([P, D], f32, tag="a")
                nc.sync.dma_start(d_t[:], samples[bass.ts(t, P), 2, :])
                nc.sync.dma_start(q_t[:], samples[bass.ts(t, P), 0, :])
                nc.sync.dma_start(a_t[:], samples[bass.ts(t, P), 1, :])
                q, a, d = q_t[:], a_t[:], d_t[:]

                # DVE: fused product + per-partition accumulate
                # (scalar_tensor_tensor; accum_out must be a standalone
                # tile — strided accum destinations crash the HW).
                dve_accs = {}
                for src0, src1, col, atag in (
                    (d, d, None, "dd1"),
                    (q, d, t, "qd1"),
                    (a, d, T + t, "ad1"),
                ):
                    jd = junk_pool.tile([P, D], f32, tag="junk_dve")
                    acc = junk_pool.tile([P, 1], f32, tag=atag)
                    nc.vector.scalar_tensor_tensor(
                        out=jd[:], in0=src0, scalar=1.0, in1=src1,
                        op0=Alu.mult, op1=Alu.mult, accum_out=acc[:],
                    )
                    dve_accs[atag] = acc
                    if col is not None:
                        nc.vector.tensor_copy(dots[:, col : col + 1], acc[:])

                # ACT: square + accumulate for the q/a norms; the norm
                # products (qq*dd, aa*dd) land per-tile so the epilogue
                # is a single fused [P, 2T] pass.
                for src0, col, atag in ((q, t, "qq1"), (a, T + t, "aa1")):
                    ja = junk_pool.tile([P, D], f32, tag="junk_act")
                    acc = junk_pool.tile([P, 1], f32, tag=atag)
                    nc.scalar.activation(
                        out=ja[:], in_=src0, func=Act.Square, accum_out=acc[:],
                    )
                    nc.vector.tensor_mul(
                        nprod[:, col : col + 1], acc[:], dve_accs["dd1"][:]
                    )

            # cos = dot / max(sqrt(nprod), EPS) in one [P, 2T] pass;
            # sqrt(v) = exp(0.5*ln(v)) keeps the whole kernel on the
            # natural_log_exp activation table (no table reload).
            inv = stats_pool.tile([P, 2 * T], f32, tag="inv")
            nc.scalar.activation(inv[:], nprod[:], Act.Ln)
            nc.scalar.activation(inv[:], inv[:], Act.Exp, scale=0.5)
            nc.vector.tensor_scalar_max(inv[:], inv[:], EPS)
            nc.vector.reciprocal(inv[:], inv[:])

            contrib = stats_pool.tile([P, 2 * T], f32, tag="contrib")
            nc.vector.tensor_mul(contrib[:], dots[:], inv[:])
            nc.vector.tensor_mul(contrib[:], contrib[:], dvb[:])

            row_sum = stats_pool.tile([P, 1], f32, tag="row_sum")
            nc.vector.reduce_sum(row_sum[:], contrib[:], axis=mybir.AxisListType.X)

            # Partition reduction via PE: [1,1] = row_sum^T @ ones.
            ones = stats_pool.tile([P, 1], f32, tag="ones")
            nc.gpsimd.memset(ones[:], 1.0)
            psum_t = psum_pool.tile([1, 1], f32, tag="psum_s")
            nc.tensor.matmul(psum_t[:], row_sum[:], ones[:], start=True, stop=True)

            # Stage the partial score, AllReduce across the 8 cores.
            partial = stats_pool.tile([1, 8], f32, tag="partial")
            nc.gpsimd.memset(partial[:], 0.0)
            nc.vector.tensor_copy(partial[0:1, 0:1], psum_t[:])

            cc_in = dram_pool.tile([1, 8], f32, tag="cc_in")
            cc_out = dram_pool.tile([1, 8], f32, tag="cc_out")
            nc.gpsimd.dma_start(cc_in[:], partial[:])
            nc.gpsimd.collective_compute(
                "AllReduce",
                Alu.add,
                replica_groups=[list(range(N_CORES))],
                ins=[cc_in[:].opt()],
                outs=[cc_out[:].opt()],
            )
            red = stats_pool.tile([1, 8], f32, tag="red")
            nc.sync.dma_start(red[:], cc_out[:])
            s = red[0:1, 0:1]

            # BCE with logits: max(s,0) - s*y + softplus(-|s|), on [1,1].
            relu_t = stats_pool.tile([1, 1], f32, tag="relu_t")
            abs_t = stats_pool.tile([1, 1], f32, tag="abs_t")
            exp_t = stats_pool.tile([1, 1], f32, tag="exp_t")
            sp_t = stats_pool.tile([1, 1], f32, tag="sp_t")
            xy_t = stats_pool.tile([1, 1], f32, tag="xy_t")
            bce_t = stats_pool.tile([1, 1], f32, tag="bce_t")
            # relu/abs on DVE: table ops on [1,1] cost ~1.3us on ACT.
            neg_t = stats_pool.tile([1, 1], f32, tag="neg_t")
            nc.vector.tensor_scalar_max(relu_t[:], s, 0.0)
            nc.vector.tensor_scalar_mul(neg_t[:], s, -1.0)
            nc.vector.tensor_max(abs_t[:], s, neg_t[:])
            # softplus(-|s|) = ln(1 + exp(-|s|)); Softplus has no HW table.
            nc.scalar.activation(exp_t[:], abs_t[:], Act.Exp, scale=-1.0)
            nc.scalar.activation(sp_t[:], exp_t[:], Act.Ln, bias=1.0)
            nc.vector.tensor_mul(xy_t[:], s, ltile[:])
            nc.vector.tensor_sub(bce_t[:], relu_t[:], xy_t[:])
            nc.vector.tensor_add(bce_t[:], bce_t[:], sp_t[:])

            nc.sync.dma_start(out[:], bce_t[:])

    nc.compile()
    return nc


def _get_program():
    if "nc" not in _CACHE:
        _CACHE["nc"] = _build_program()
    return _CACHE["nc"]


def kernel(samples, labels, D_v1, D_v2):
    samples = np.asarray(samples, dtype=np.float32)
    labels = np.asarray(labels, dtype=np.float32)
    D_v1 = np.asarray(D_v1, dtype=np.float32)
    D_v2 = np.asarray(D_v2, dtype=np.float32)
    assert samples.shape == (B, 3, D), samples.shape

    nc = _get_program()

    in_maps = []
    for c in range(N_CORES):
        sl = slice(c * BS, (c + 1) * BS)
        in_maps.append(
            {
                "samples": np.ascontiguousarray(samples[sl]),
                "labels": labels,
                "dv1": np.ascontiguousarray(D_v1[sl]),
                "dv2": np.ascontiguousarray(D_v2[sl]),
            }
        )

    res = bass_utils.run_bass_kernel_spmd(nc, in_maps, core_ids=list(range(N_CORES)))
    _CACHE["last_results"] = res
    return np.asarray(res.results[0]["out"], dtype=np.float32).reshape(())
